# revision 41
# baseline (speedup 1.0000x reference)
"""Trainium2 Bass kernel for nn_AsymmetricProjectedLinear (8 NeuronCores).

Reference computes out = x @ W_large^T with
    W_large = (A_out @ B_out) @ W_small @ (A_in @ B_in)^T    [4096, 4096]

W_large (180 GFLOP naive) is never materialized. Factored (~4.4 GFLOP):
    M   = B_in @ W_small^T @ B_out^T            [64, 64]   (tiny)
    out = ((x @ A_in) @ M) @ A_out^T            [4096t, 4096]

Sharding: tokens (B*S = 4096) split 512/core across 8 cores; weights
replicated. (A 16KB AllReduce for M was tried and costs ~50us wall on
this runtime -- far above its nominal floor -- so every core redundantly
computes M from the full W_small instead.) Host work is layout-only
(transpose/pack/slice/dtype-cast); all FLOPs run on the NeuronCores.

Per-core device pipeline (T = 512 local tokens, two token blocks of 256):
  prework:  G = B_out @ W_small [64, 1024]; G^T via PE transpose;
            M = B_in @ G^T [64, 64].  W lands first on the wire, so this
            runs during the x stream and doubles as PE warmup.
  per token block b:
    stage 1:  u1T_b = A_in^T @ xT_b              [64, 256]  (K=d, 32 MMs)
    stage 2:  t2T_b = lhsT=M, rhs=u1T_b; dup into both partition halves
    stage 5:  out_b = t2_b @ A_out^T, row-packed 2x: the two 128-token
              slices run concurrently in the two PE row halves (K=64
              each), their PSUM tiles drained by DVE + ACT in parallel.
  Stage-5(b0) op-pairs are interleaved with stage-1(b1) matmuls so
  out(b0) DMAs spread across the x(b1) stream and the wire never idles.

Empirical notes driving the design (from perfetto/NTFF traces):
  - One HWDGE ring saturates at ~250 GB/s; Sync + Scalar rings together
    reach ~430. Every large transfer is split/alternated across both.
  - Engines execute their streams in emission order; emission order IS
    the schedule. Ring order = byte arrival time (FIFO).
  - f32r streams the PE at ~4 cyc/row (bus-limited) and its self-loading
    LDWEIGHTS cannot be amortized, making PE time (~64us) exceed the DMA
    wire (~53us). bf16 halves both and is the default.
  - Compute dtype bf16 gives rel_err 5.5e-3 vs the f32 reference (the
    harness gate for this problem family is rel_err < 2e-2); KERNEL_DT=
    f32r selects a 4-byte path at rel_err 3.2e-4, ~1.45x slower.

All matmul operands are fed in natural [K-on-partitions] layout via
host-side packing, so no on-chip transposes of x are needed.
"""

import os

import numpy as np

import concourse.bass as bass
import concourse.mybir as mybir
import concourse.tile as tile
from concourse import bacc
from concourse.bass_utils import run_bass_kernel_spmd

N_CORES = 8
Bsz, S, D = 2, 2048, 4096
TOK = Bsz * S          # 4096 tokens
T = TOK // N_CORES     # 512 tokens per core
TB = 256               # tokens per pipeline block (f32r needs N >= 256)
NBLK = T // TB         # 2 blocks
RANK = 64
DS = 1024              # d_small

F32 = mybir.dt.float32
# Compute dtype for all TensorEngine-facing tensors.
#  - "bf16": 1 cyc/row PE streaming, half DMA bytes; rel err ~3e-3
#    (gate for this harness family is rel_err < 2e-2)
#  - "f32r": 4-byte single-pass, ~4 cyc/row measured; rel err ~3e-4
#  - "f32":  full fp32 (2 half-speed passes); slowest
DTYPE_MODE = os.environ.get("KERNEL_DT", "bf16")
MM_DT = {"bf16": mybir.dt.bfloat16,
         "f32r": mybir.dt.float32r,
         "f32": mybir.dt.float32}[DTYPE_MODE]
# Output storage dtype: bf16 mode also writes bf16 output (host upcasts);
# the rounding is the same error class as the bf16 compute itself.
OUT_DT = mybir.dt.bfloat16 if DTYPE_MODE == "bf16" else F32

_nc_cache = {}


def build():
    key = (MM_DT, OUT_DT)
    if key in _nc_cache:
        return _nc_cache[key]
    nc = bacc.Bacc("TRN2", target_bir_lowering=False, debug=False,
                   num_devices=N_CORES)

    # Inputs, all pre-packed on host into direct SBUF layouts.
    # x_p: [NBLK][128, 32 d-tiles * TB]
    x_p = nc.dram_tensor("x_p", [NBLK, 128, 32 * TB], MM_DT, kind="ExternalInput")
    # b_outT | b_inT | a_in packed into one tensor -> single front DMA
    wsm_p = nc.dram_tensor("wsm_p", [128, 8 * RANK * 2 + 32 * RANK], MM_DT,
                           kind="ExternalInput")
    a_outT = nc.dram_tensor("a_outT", [RANK, D], MM_DT, kind="ExternalInput")
    w_p = nc.dram_tensor("w_p", [128, 8 * DS], MM_DT, kind="ExternalInput")
    ident = nc.dram_tensor("ident", [RANK, RANK], MM_DT, kind="ExternalInput")
    out = nc.dram_tensor("out", [T, D], OUT_DT, kind="ExternalOutput")

    with tile.TileContext(nc) as tc:
        with (
            tc.tile_pool(name="const", bufs=1) as cpool,
            tc.tile_pool(name="xin", bufs=8 if DTYPE_MODE == "bf16" else 4) as xpool,
            tc.tile_pool(name="outp", bufs=4) as opool,
            tc.tile_pool(name="interm", bufs=2) as ipool,
            tc.tile_pool(name="ps_a", bufs=2, space="PSUM") as ps_a,
            tc.tile_pool(name="ps_o", bufs=6, space="PSUM") as ps_o,
        ):
            # ---- input streams, interleaved across BOTH HWDGE rings --
            # Ring order matters twice over: each HWDGE ring drains FIFO
            # (so byte position = arrival time), and small gating weights
            # must land before the big streams that hide them.
            wsm_s = cpool.tile([128, 8 * RANK * 2 + 32 * RANK], MM_DT)
            b_outT_s = wsm_s[:, 0:8 * RANK]
            b_inT_s = wsm_s[:, 8 * RANK:16 * RANK]
            a_in_s = wsm_s[:, 16 * RANK:]
            ident_s = cpool.tile([RANK, RANK], MM_DT)
            a_outT_s = cpool.tile([128, D], MM_DT)
            x_tiles = [[None] * 4 for _ in range(NBLK)]
            w_tiles = [None] * 8

            def _x_chunk(eng, b, p):
                xt = xpool.tile([128, 8 * TB], MM_DT, tag=f"xc{p % 2}")
                eng.dma_start(
                    out=xt[:, :],
                    in_=x_p.ap()[b, :, p * 8 * TB:(p + 1) * 8 * TB],
                )
                x_tiles[b][p] = xt

            def _w_chunk(eng, j0):
                for j in (j0, j0 + 1):
                    wt = cpool.tile([128, DS], MM_DT, tag=f"w{j}")
                    eng.dma_start(out=wt[:, :],
                                  in_=w_p.ap()[:, j * DS:(j + 1) * DS])
                    w_tiles[j] = wt

            # sync ring:   b_outT, ident, W[0,1,4,5], x pieces (even), a_outT lo
            # scalar ring: a_in, b_inT, W[2,3,6,7], x pieces (odd), a_outT hi
            # W lands FIRST: prework runs at ~14-18us and doubles as PE
            # warmup (HAM); x(b0) streams in 1MB pieces consumed as they
            # land so the PE never idles long enough to re-throttle.
            # only b_outT (G's lhsT) must precede W; b_inT|a_in follow
            # after W so the G->M prework chain unblocks ~4us earlier.
            nc.sync.dma_start(out=wsm_s[:, :8 * RANK], in_=wsm_p.ap()[:, :8 * RANK])
            nc.scalar.dma_start(out=ident_s[:, :], in_=ident.ap())
            _w_chunk(nc.sync, 0)
            _w_chunk(nc.scalar, 2)
            _w_chunk(nc.sync, 4)
            _w_chunk(nc.scalar, 6)
            nc.scalar.dma_start(out=wsm_s[:, 8 * RANK:], in_=wsm_p.ap()[:, 8 * RANK:])
            for p in range(4):                     # x(b0) 1MB pieces
                _x_chunk([nc.sync, nc.scalar][p % 2], 0, p)
            nc.sync.dma_start(out=a_outT_s[:RANK, :2048], in_=a_outT.ap()[:, :2048])
            nc.scalar.dma_start(out=a_outT_s[:RANK, 2048:], in_=a_outT.ap()[:, 2048:])
            nc.gpsimd.dma_start(out=a_outT_s[RANK:, :], in_=a_outT_s[:RANK, :])
            for p in range(4):                     # x(b1) 1MB pieces
                _x_chunk([nc.sync, nc.scalar][p % 2], 1, p)

            def stage1_mms(b, mlo, mhi, u1_ps):
                for m in range(mlo, mhi):
                    xt = x_tiles[b][m // 8]
                    sl = m % 8
                    nc.tensor.matmul(
                        u1_ps[:, :],
                        a_in_s[:, m * RANK:(m + 1) * RANK],
                        xt[:, sl * TB:(sl + 1) * TB],
                        start=(m == 0), stop=(m == 31),
                    )

            def stage1_close(u1_ps):
                u1_s = ipool.tile([RANK, TB], MM_DT, tag="u1")
                nc.vector.tensor_copy(u1_s[:, :], u1_ps[:, :])
                return u1_s

            def prework():
                # G = B_out @ W_small  [64, 1024], j-major: both halves'
                # accumulation chains advance as each W j-tile lands, so G
                # completes ~1 MM after the last W chunk instead of
                # re-walking all j for the second half.
                g_s = cpool.tile([RANK, DS], MM_DT)
                g_ps0 = ps_a.tile([RANK, 512], F32, tag="ps_small")
                g_ps1 = ps_a.tile([RANK, 512], F32, tag="ps_small")
                g_ps = (g_ps0, g_ps1)
                for j in range(8):
                    for h in range(2):
                        nc.tensor.matmul(
                            g_ps[h][:, :],
                            b_outT_s[:, j * RANK:(j + 1) * RANK],
                            w_tiles[j][:, h * 512:(h + 1) * 512],
                            start=(j == 0), stop=(j == 7),
                        )
                for h in range(2):
                    nc.vector.tensor_copy(
                        g_s[:, h * 512:(h + 1) * 512], g_ps[h][:, :])
                # G^T via PE transpose
                gT_s = cpool.tile([128, 8 * RANK], MM_DT)
                for it in range(8):
                    gt_ps = ps_a.tile([128, RANK], MM_DT, tag="ps_small")
                    nc.tensor.transpose(
                        gt_ps[:, :], g_s[:, it * 128:(it + 1) * 128],
                        ident_s[:, :])
                    nc.vector.tensor_copy(
                        gT_s[:, it * RANK:(it + 1) * RANK], gt_ps[:, :])
                # M = B_in @ G^T  [64, 64]
                m_ps = ps_a.tile([RANK, RANK], F32, tag="ps_small")
                for it in range(8):
                    nc.tensor.matmul(
                        m_ps[:, :],
                        b_inT_s[:, it * RANK:(it + 1) * RANK],
                        gT_s[:, it * RANK:(it + 1) * RANK],
                        start=(it == 0), stop=(it == 7),
                    )
                m_s = cpool.tile([RANK, RANK], MM_DT)
                nc.vector.tensor_copy(m_s[:, :], m_ps[:, :])
                return m_s

            def stage2(b, u1_s, m_s):
                # t2T_b; duplicated into both partition halves (row-packed
                # stage 5 needs partitions 0-63 and 64-127; cross-partition
                # moves need a DMA, DVE lanes are fixed)
                t2_ps = ps_a.tile([RANK, TB], F32, tag="ps_small")
                nc.tensor.matmul(
                    t2_ps[:, :], m_s[:, :], u1_s[:, :], start=True, stop=True,
                )
                t2_lo = ipool.tile([RANK, TB], MM_DT, tag="t2lo")
                t2_hi = ipool.tile([128, TB], MM_DT, tag="t2hi")
                nc.vector.tensor_copy(t2_lo[:, :], t2_ps[:, :])
                nc.gpsimd.dma_start(out=t2_hi[RANK:, :], in_=t2_lo[:, :])
                return t2_lo, t2_hi

            def stage5_op(b, t2, op):
                t2_lo, t2_hi = t2
                # one o-pair: out rows [b*TB, (b+1)*TB), cols op*1024 +:1024;
                # t-slices (0, 1) of the block row-packed into both halves.
                o_t0 = opool.tile([128, 1024], OUT_DT, tag="o_lo")
                o_t1 = opool.tile([128, 1024], OUT_DT, tag="o_hi")
                for oi in range(2):
                    o = op * 2 + oi
                    po0 = ps_o.tile([128, 512], F32, tag="ps_out")
                    po1 = ps_o.tile([128, 512], F32, tag="ps_out")
                    nc.tensor.matmul(
                        po0[:, :],
                        t2_lo[:, 0:128],
                        a_outT_s[:RANK, o * 512:(o + 1) * 512],
                        start=True, stop=True,
                    )
                    nc.tensor.matmul(
                        po1[:, :],
                        t2_hi[RANK:, 128:256],
                        a_outT_s[RANK:, o * 512:(o + 1) * 512],
                        start=True, stop=True,
                    )
                    nc.vector.tensor_copy(
                        o_t0[:, oi * 512:(oi + 1) * 512], po0[:, :])
                    nc.scalar.copy(
                        o_t1[:, oi * 512:(oi + 1) * 512], po1[:, :])
                r0 = b * TB
                for (row, o_t), eng in (((r0, o_t0), nc.sync),
                                        ((r0 + 128, o_t1), nc.scalar)):
                    eng.dma_start(
                        out=out.ap()[row:row + 128,
                                     op * 1024:(op + 1) * 1024],
                        in_=o_t[:, :],
                    )

            # PE stream in data-arrival order: W first (prework = warmup),
            # then x(b0), then x(b1). Stage-5(b0) op-pairs interleave with
            # stage-1(b1) matmuls: out(b0) DMAs spread out so the
            # psum->copy->out-tile recycling never stalls stage 5(b1).
            m_s = prework()
            u1p_b0 = ps_a.tile([RANK, TB], F32, tag="ps_small")
            stage1_mms(0, 0, 32, u1p_b0)
            u1_b0 = stage1_close(u1p_b0)
            t2_b0 = stage2(0, u1_b0, m_s)
            u1p_b1 = ps_a.tile([RANK, TB], F32, tag="ps_small")
            for op in range(4):
                stage5_op(0, t2_b0, op)
                stage1_mms(1, op * 8, (op + 1) * 8, u1p_b1)
            u1_b1 = stage1_close(u1p_b1)
            t2_b1 = stage2(1, u1_b1, m_s)
            for op in range(4):
                stage5_op(1, t2_b1, op)

    nc.compile()
    _nc_cache[key] = nc
    return nc


def _prep_in_maps(x, W_small, A_out, B_out, A_in, B_in):
    import ml_dtypes
    f = (ml_dtypes.bfloat16 if DTYPE_MODE == "bf16" else np.float32)
    x2 = np.asarray(x, dtype=f).reshape(TOK, D)
    a_in_p = np.ascontiguousarray(
        np.asarray(A_in, f).reshape(32, 128, RANK).transpose(1, 0, 2)
    ).reshape(128, 32 * RANK)
    a_outT = np.ascontiguousarray(np.asarray(A_out, f).T)
    b_inT_p = np.ascontiguousarray(
        np.asarray(B_in, f).T.reshape(8, 128, RANK).transpose(1, 0, 2)
    ).reshape(128, 8 * RANK)
    b_outT_p = np.ascontiguousarray(
        np.asarray(B_out, f).T.reshape(8, 128, RANK).transpose(1, 0, 2)
    ).reshape(128, 8 * RANK)
    wsm_p = np.ascontiguousarray(
        np.concatenate([b_outT_p, b_inT_p, a_in_p], axis=1))
    w_p = np.ascontiguousarray(
        np.asarray(W_small, f).reshape(8, 128, DS).transpose(1, 0, 2)
    ).reshape(128, 8 * DS)
    ident = np.eye(RANK, dtype=f)

    shared = {
        "wsm_p": wsm_p, "a_outT": a_outT, "w_p": w_p, "ident": ident,
    }
    in_maps = []
    for c in range(N_CORES):
        xs = x2[c * T:(c + 1) * T, :]            # [T, 4096]
        # block b: tokens [b*TB, (b+1)*TB) -> 32 d-tiles [128, TB] packed
        xp = np.ascontiguousarray(
            xs.T                                  # [4096, T]
            .reshape(32, 128, NBLK, TB)           # d-tile, p, blk, t
            .transpose(2, 1, 0, 3)                # blk, p, d-tile, t
        ).reshape(NBLK, 128, 32 * TB)
        in_maps.append({"x_p": xp, **shared})
    return in_maps


def _run(inputs, trace=False):
    nc = build()
    in_maps = _prep_in_maps(**inputs)
    res = run_bass_kernel_spmd(
        nc, in_maps, core_ids=list(range(N_CORES)), trace=trace
    )
    out = np.concatenate(
        [np.asarray(res.results[c]["out"], dtype=np.float32)
         for c in range(N_CORES)], axis=0
    ).reshape(Bsz, S, D)
    return out, res


def kernel(**inputs) -> np.ndarray:
    out, _ = _run(inputs, trace=False)
    return out
